# revision 2
# baseline (speedup 1.0000x reference)
"""Causal self-attention (S=2048, D=1024, H=16) on 8 Trainium2 NeuronCores.

Sharding: tensor-parallel over heads. Core c owns heads 2c, 2c+1:
  - computes q,k (transposed layout) and v (natural layout) for its 128
    qkv-columns from the full hidden_states,
  - runs causal attention for its 2 heads (attT = K.Q^T blocks, exp via
    ScalarE, denominators via a ones-column in the PV matmul),
  - contracts its 128 head-dims against its W_proj row-slice,
  - outputs a partial [S, D] product; the host sums the 8 partials and
    adds b_proj.
"""

import math

import numpy as np

import concourse.bacc as bacc
import concourse.mybir as mybir
import concourse.tile as tile
from concourse.bass_utils import run_bass_kernel_spmd

S, D, H = 2048, 1024, 16
HS = D // H  # 64 head size
P = 128
NCORES = 8
HPC = H // NCORES  # 2 heads per core
CD = HPC * HS  # 128 per-core head dims
KO = D // P  # 8 contraction tiles for the projections
NQC = S // 512  # 4 query chunks
NSC = S // P  # 16 sequence chunks of 128
SCALE = 1.0 / math.sqrt(S)

F32 = mybir.dt.float32
BF16 = mybir.dt.bfloat16

# Per-phase matmul input dtypes (device side). Host sends matching arrays.
DT_P1 = F32  # hsT / w_qk / w_v (qkv projection)
DT_QK = F32  # qT/kT tiles feeding the logits matmul
DT_PV = F32  # exp(att) and v tiles feeding the PV matmul
DT_P3 = F32  # attn output and w_proj feeding the final projection

_NP = {F32: np.float32}
try:
    import ml_dtypes

    _NP[BF16] = ml_dtypes.bfloat16
except ImportError:  # pragma: no cover
    pass


def _build():
    nc = bacc.Bacc(
        "TRN2", target_bir_lowering=False, debug=False, num_devices=NCORES
    )

    hsT = nc.dram_tensor("hsT", [D, S], DT_P1, kind="ExternalInput")
    w_qk = nc.dram_tensor("w_qk", [D, 2 * P], DT_P1, kind="ExternalInput")
    b_qk = nc.dram_tensor("b_qk", [P, 2], F32, kind="ExternalInput")
    w_v = nc.dram_tensor("w_v", [D, CD], DT_P1, kind="ExternalInput")
    b_v = nc.dram_tensor("b_v", [1, CD], F32, kind="ExternalInput")
    w_p = nc.dram_tensor("w_p", [CD, D], DT_P3, kind="ExternalInput")
    msk = nc.dram_tensor("msk", [P, 896], DT_PV, kind="ExternalInput")
    out = nc.dram_tensor("out", [S, D], F32, kind="ExternalOutput")

    with tile.TileContext(nc) as tc:
        from contextlib import ExitStack

        with ExitStack() as ctx:
            const = ctx.enter_context(tc.tile_pool(name="const", bufs=1))
            work = ctx.enter_context(tc.tile_pool(name="work", bufs=2))
            pm = ctx.enter_context(tc.tile_pool(name="pm", bufs=2, space="PSUM"))
            pa = ctx.enter_context(tc.tile_pool(name="pa", bufs=1, space="PSUM"))
            po = ctx.enter_context(tc.tile_pool(name="po", bufs=1, space="PSUM"))

            # ---- constant loads -------------------------------------------------
            hsT_sb = const.tile([P, KO, S], DT_P1, tag="hsT", name="hsT_sb")
            for o in range(KO):
                nc.sync.dma_start(out=hsT_sb[:, o, :], in_=hsT.ap()[o * P : (o + 1) * P, :])
            wqk_sb = const.tile([P, KO, 2 * P], DT_P1, tag="wqk", name="wqk_sb")
            nc.sync.dma_start(
                out=wqk_sb, in_=w_qk.ap().rearrange("(o p) m -> p o m", p=P)
            )
            wv_sb = const.tile([P, KO, CD], DT_P1, tag="wv", name="wv_sb")
            nc.sync.dma_start(
                out=wv_sb, in_=w_v.ap().rearrange("(o p) m -> p o m", p=P)
            )
            bqk_sb = const.tile([P, 2], F32, tag="bqk", name="bqk_sb")
            nc.sync.dma_start(out=bqk_sb, in_=b_qk.ap())
            bv_sb = const.tile([1, CD], F32, tag="bv", name="bv_sb")
            nc.sync.dma_start(out=bv_sb, in_=b_v.ap())
            wp_sb = const.tile([P, D], DT_P3, tag="wp", name="wp_sb")
            nc.sync.dma_start(out=wp_sb, in_=w_p.ap())
            msk_sb = const.tile([P, 896], DT_PV, tag="msk", name="msk_sb")
            nc.sync.dma_start(out=msk_sb, in_=msk.ap())

            ones_sb = const.tile([1, P], F32, tag="ones", name="ones_sb")
            nc.vector.memset(ones_sb, 1.0)

            qkT_sb = const.tile([P, 2, S], DT_QK, tag="qkT", name="qkT_sb")
            v_sb = []
            for h in range(HPC):
                vt = const.tile([P, NSC, HS + 1], DT_PV, tag=f"v{h}", name=f"v{h}_sb")
                nc.vector.memset(vt[:, :, HS : HS + 1], 1.0)
                v_sb.append(vt)
            attn_sb = const.tile([P, S], DT_P3, tag="attn", name="attn_sb")

            # ---- phase 1: qT, kT ([j, s] layout) --------------------------------
            for n in range(NQC):
                for m in range(2):
                    ps_qk = pm.tile([P, 512], F32, tag="mm", name="ps_qk")
                    for o in range(KO):
                        nc.tensor.matmul(
                            ps_qk,
                            lhsT=wqk_sb[:, o, m * P : (m + 1) * P],
                            rhs=hsT_sb[:, o, n * 512 : (n + 1) * 512],
                            start=(o == 0),
                            stop=(o == KO - 1),
                        )
                    nc.any.tensor_scalar_add(
                        out=qkT_sb[:, m, n * 512 : (n + 1) * 512],
                        in0=ps_qk,
                        scalar1=bqk_sb[:, m : m + 1],
                    )

            # ---- phase 1: v (natural [s, j] layout) -----------------------------
            for sc in range(NSC):
                ps_v = pm.tile([P, P], F32, tag="mm", name="ps_v")
                for o in range(KO):
                    nc.tensor.matmul(
                        ps_v,
                        lhsT=hsT_sb[:, o, sc * P : (sc + 1) * P],
                        rhs=wv_sb[:, o, :],
                        start=(o == 0),
                        stop=False,
                    )
                nc.tensor.matmul(
                    ps_v, lhsT=ones_sb[:, :P], rhs=bv_sb, start=False, stop=True
                )
                for h in range(HPC):
                    nc.any.tensor_copy(
                        out=v_sb[h][:, sc, 0:HS], in_=ps_v[:, h * HS : (h + 1) * HS]
                    )

            # ---- phase 2: causal attention per head -----------------------------
            for qc in range(NQC):
                ps_o = [
                    po.tile([P, 512], F32, tag=f"o{h}", name=f"ps_o{h}")
                    for h in range(HPC)
                ]
                nkb = 4 * (qc + 1)  # 128-wide key blocks in the causal span
                for g in range(nkb // 2):
                    for h in range(HPC):
                        ps_att = pa.tile([P, 2, 512], F32, tag=f"a{h}", name="ps_att")
                        for j in range(2):
                            kb = 2 * g + j
                            nc.tensor.matmul(
                                ps_att[:, j, :],
                                lhsT=qkT_sb[h * HS : (h + 1) * HS, 1, kb * P : (kb + 1) * P],
                                rhs=qkT_sb[h * HS : (h + 1) * HS, 0, qc * 512 : (qc + 1) * 512],
                                start=True,
                                stop=True,
                            )
                        p_exp = work.tile(
                            [P, 2, 512], DT_PV, tag=f"pe{h}", bufs=2, name="p_exp"
                        )
                        nc.scalar.activation(
                            out=p_exp,
                            in_=ps_att,
                            func=mybir.ActivationFunctionType.Exp,
                            scale=SCALE,
                        )
                        for j in range(2):
                            jj = 2 * g + j - 4 * qc
                            if jj >= 0:  # diagonal block: causal 0/1 mask
                                off = 384 - 128 * jj
                                nc.vector.tensor_mul(
                                    out=p_exp[:, j, :],
                                    in0=p_exp[:, j, :],
                                    in1=msk_sb[:, off : off + 512],
                                )
                        for j in range(2):
                            kb = 2 * g + j
                            nc.tensor.matmul(
                                ps_o[h][0 : HS + 1, :],
                                lhsT=v_sb[h][:, kb, :],
                                rhs=p_exp[:, j, :],
                                start=(kb == 0),
                                stop=(kb == nkb - 1),
                            )
                for h in range(HPC):
                    r_sb = work.tile([1, 512], F32, tag=f"r{h}", bufs=2, name="r_sb")
                    nc.vector.reciprocal(out=r_sb, in_=ps_o[h][HS : HS + 1, :])
                    ps_bc = pm.tile([HS, 512], F32, tag="mm", name="ps_bc")
                    nc.tensor.matmul(
                        ps_bc, lhsT=ones_sb[:, 0:HS], rhs=r_sb, start=True, stop=True
                    )
                    rb_sb = work.tile([HS, 512], F32, tag=f"rb{h}", bufs=2, name="rb_sb")
                    nc.any.tensor_copy(out=rb_sb, in_=ps_bc)
                    nc.vector.tensor_mul(
                        out=attn_sb[h * HS : (h + 1) * HS, qc * 512 : (qc + 1) * 512],
                        in0=ps_o[h][0:HS, :],
                        in1=rb_sb,
                    )

            # ---- phase 3: partial output projection -----------------------------
            for sc in range(NSC):
                out_t = work.tile([P, D], F32, tag="out", bufs=2, name="out_t")
                for dc in range(2):
                    ps_p3 = pm.tile([P, 512], F32, tag="mm", name="ps_p3")
                    nc.tensor.matmul(
                        ps_p3,
                        lhsT=attn_sb[:, sc * P : (sc + 1) * P],
                        rhs=wp_sb[:, dc * 512 : (dc + 1) * 512],
                        start=True,
                        stop=True,
                    )
                    nc.any.tensor_copy(out=out_t[:, dc * 512 : (dc + 1) * 512], in_=ps_p3)
                nc.sync.dma_start(out=out.ap()[sc * P : (sc + 1) * P, :], in_=out_t)

    nc.compile()
    return nc


_NC = None


def _get_nc():
    global _NC
    if _NC is None:
        _NC = _build()
    return _NC


def _np_dt(dt):
    return _NP[dt]


def prepare_inputs(hidden_states, W_attn, b_attn, W_proj, b_proj):
    hs = np.asarray(hidden_states, dtype=np.float32)
    Wa = np.asarray(W_attn, dtype=np.float32)
    ba = np.asarray(b_attn, dtype=np.float32)
    Wp = np.asarray(W_proj, dtype=np.float32)

    hsT = np.ascontiguousarray(hs.T).astype(_np_dt(DT_P1))
    pcol = np.arange(P)[:, None]
    ccol = np.arange(896)[None, :]
    msk = (pcol <= ccol - 384).astype(_np_dt(DT_PV))

    in_maps = []
    for c in range(NCORES):
        q0 = c * CD
        wq = Wa[:, q0 : q0 + CD]
        wk = Wa[:, D + q0 : D + q0 + CD]
        wv = Wa[:, 2 * D + q0 : 2 * D + q0 + CD]
        bq = ba[q0 : q0 + CD]
        bk = ba[D + q0 : D + q0 + CD]
        bv = ba[2 * D + q0 : 2 * D + q0 + CD]
        in_maps.append(
            {
                "hsT": hsT,
                "w_qk": np.ascontiguousarray(
                    np.concatenate([wq, wk], axis=1)
                ).astype(_np_dt(DT_P1)),
                "b_qk": np.ascontiguousarray(np.stack([bq, bk], axis=1)).astype(
                    np.float32
                ),
                "w_v": np.ascontiguousarray(wv).astype(_np_dt(DT_P1)),
                "b_v": np.ascontiguousarray(bv.reshape(1, CD)).astype(np.float32),
                "w_p": np.ascontiguousarray(Wp[q0 : q0 + CD, :]).astype(
                    _np_dt(DT_P3)
                ),
                "msk": msk,
            }
        )
    return in_maps


def run(inputs, trace=False):
    """Build+run the sharded kernel. Returns (full_output, BassKernelResults)."""
    in_maps = prepare_inputs(**inputs)
    nc = _get_nc()
    res = run_bass_kernel_spmd(
        nc, in_maps, core_ids=list(range(NCORES)), trace=trace
    )
    acc = np.zeros((S, D), dtype=np.float32)
    for c in range(NCORES):
        acc += res.results[c]["out"]
    acc += np.asarray(inputs["b_proj"], dtype=np.float32)
    return acc, res


def kernel(**inputs):
    out, _ = run(inputs, trace=False)
    return out


# revision 8
# speedup vs baseline: 1.7926x; 1.7926x over previous
"""Causal self-attention (S=2048, D=1024, H=16) on 8 Trainium2 NeuronCores.

Sharding: tensor-parallel over heads. Core c owns heads 2c, 2c+1:
  - computes qT/kT/vT for its 128 qkv-columns from the full hidden_states
    (contraction layouts; vT is PE-transposed back to natural [s, j]),
  - runs causal attention for its 2 heads (attT = K.Q^T blocks, exp via
    ScalarE, denominators via a ones-column in the PV matmul),
  - contracts its 128 head-dims against its W_proj row-slice,
  - outputs a partial [S, D] product; the host sums the 8 partials and
    adds b_proj.

Matmuls run in float32r (fp32 storage, single-pass PE) where enabled —
4x the fp32 matmul rate.
"""

import math
from contextlib import ExitStack

import numpy as np

import concourse.bacc as bacc
import concourse.mybir as mybir
import concourse.tile as tile
from concourse.bass_utils import run_bass_kernel_spmd

S, D, H = 2048, 1024, 16
HS = D // H  # 64 head size
P = 128
NCORES = 8
HPC = H // NCORES  # 2 heads per core
CD = HPC * HS  # 128 per-core head dims
KO = D // P  # 8 contraction tiles for the projections
NQC = S // 512  # 4 query chunks
NSC = S // P  # 16 sequence chunks of 128
SCALE = 1.0 / math.sqrt(S)

F32 = mybir.dt.float32
F32R = mybir.dt.float32r

# float32r per phase (empirically validated against the fp32 reference)
R_P1 = True  # q/k/v projections
R_ATT = True  # logits (K.Q^T)
R_PV = True  # exp(att) @ v
R_P3 = True  # output projection


def _r(ap, on=True):
    return ap.bitcast(F32R) if on else ap


def _build():
    nc = bacc.Bacc(
        "TRN2", target_bir_lowering=False, debug=False, num_devices=NCORES
    )

    hsT = nc.dram_tensor("hsT", [D, S], F32R, kind="ExternalInput")
    w_qkv = nc.dram_tensor("w_qkv", [D, 3 * P], F32R, kind="ExternalInput")
    b_qkv = nc.dram_tensor("b_qkv", [P, 3], F32, kind="ExternalInput")
    w_p = nc.dram_tensor("w_p", [CD, D], F32R, kind="ExternalInput")
    msk = nc.dram_tensor("msk", [P, 896], F32R, kind="ExternalInput")
    iden = nc.dram_tensor("iden", [P, P], F32R, kind="ExternalInput")
    vones = nc.dram_tensor("vones", [P, NSC], F32R, kind="ExternalInput")
    out = nc.dram_tensor("out", [S, D], F32, kind="ExternalOutput")

    with (
        tile.TileContext(nc) as tc,
        ExitStack() as ctx,
        nc.allow_low_precision(reason="float32r matmul pipeline"),
    ):
        const = ctx.enter_context(tc.tile_pool(name="const", bufs=1))
        work = ctx.enter_context(tc.tile_pool(name="work", bufs=2))
        pm = ctx.enter_context(tc.tile_pool(name="pm", bufs=2, space="PSUM"))
        pa = ctx.enter_context(tc.tile_pool(name="pa", bufs=1, space="PSUM"))
        po = ctx.enter_context(tc.tile_pool(name="po", bufs=1, space="PSUM"))

        # ---- constant loads -------------------------------------------------
        hsT_sb = const.tile([P, KO, S], F32R, tag="hsT", name="hsT_sb")
        for o in range(KO):
            nc.sync.dma_start(out=hsT_sb[:, o, :], in_=hsT.ap()[o * P : (o + 1) * P, :])
        wqkv_sb = const.tile([P, KO, 3 * P], F32R, tag="wqkv", name="wqkv_sb")
        nc.sync.dma_start(
            out=wqkv_sb, in_=w_qkv.ap().rearrange("(o p) m -> p o m", p=P)
        )
        bqkv_sb = const.tile([P, 3], F32, tag="bqkv", name="bqkv_sb")
        nc.sync.dma_start(out=bqkv_sb, in_=b_qkv.ap())
        wp_sb = const.tile([P, D], F32R, tag="wp", name="wp_sb")
        nc.sync.dma_start(out=wp_sb, in_=w_p.ap())
        msk_sb = const.tile([P, 896], F32R, tag="msk", name="msk_sb")
        nc.sync.dma_start(out=msk_sb, in_=msk.ap())

        ident = const.tile([P, P], F32R, tag="ident", name="ident")
        nc.sync.dma_start(out=ident, in_=iden.ap())

        qkT_sb = const.tile([P, 2, S], F32R, tag="qkT", name="qkT_sb")
        vT_sb = const.tile([P, S], F32R, tag="vT", name="vT_sb")
        v_sb = []
        for h in range(HPC):
            vt = const.tile([P, NSC, HS + 1], F32R, tag=f"v{h}", name=f"v{h}_sb")
            nc.sync.dma_start(out=vt[:, :, HS], in_=vones.ap())
            v_sb.append(vt)
        attn_sb = const.tile([P, S], F32R, tag="attn", name="attn_sb")

        # ---- phase 1: qT, kT, vT ([j, s] layout) ----------------------------
        for n in range(NQC):
            for m in range(3):
                ps_qkv = pm.tile([P, 512], F32, tag="mm", name="ps_qkv")
                for o in range(KO):
                    nc.tensor.matmul(
                        ps_qkv,
                        lhsT=_r(wqkv_sb[:, o, m * P : (m + 1) * P], R_P1),
                        rhs=_r(hsT_sb[:, o, n * 512 : (n + 1) * 512], R_P1),
                        start=(o == 0),
                        stop=(o == KO - 1),
                    )
                dst = (
                    qkT_sb[:, m, n * 512 : (n + 1) * 512]
                    if m < 2
                    else vT_sb[:, n * 512 : (n + 1) * 512]
                )
                nc.any.tensor_scalar_add(
                    out=dst, in0=ps_qkv, scalar1=bqkv_sb[:, m : m + 1]
                )

        # ---- phase 1b: transpose vT -> natural v per 128-chunk --------------
        for sc in range(NSC):
            ps_t = pm.tile([P, P], F32R, tag="mm", name="ps_t")
            nc.tensor.transpose(ps_t, vT_sb[:, sc * P : (sc + 1) * P], ident)
            for h in range(HPC):
                nc.any.tensor_copy(
                    out=v_sb[h][:, sc, 0:HS], in_=ps_t[:, h * HS : (h + 1) * HS]
                )

        # ---- phase 2: causal attention per head -----------------------------
        for qc in range(NQC):
            ps_o = [
                po.tile([P, 512], F32, tag=f"o{h}", name=f"ps_o{h}")
                for h in range(HPC)
            ]
            nkb = 4 * (qc + 1)  # 128-wide key blocks in the causal span
            for g in range(nkb // 2):
                for h in range(HPC):
                    ps_att = pa.tile([P, 2, 512], F32, tag=f"a{h}", name="ps_att")
                    for j in range(2):
                        kb = 2 * g + j
                        nc.tensor.matmul(
                            ps_att[:, j, :],
                            lhsT=_r(
                                qkT_sb[h * HS : (h + 1) * HS, 1, kb * P : (kb + 1) * P],
                                R_ATT,
                            ),
                            rhs=_r(
                                qkT_sb[h * HS : (h + 1) * HS, 0, qc * 512 : (qc + 1) * 512],
                                R_ATT,
                            ),
                            start=True,
                            stop=True,
                        )
                    p_exp = work.tile(
                        [P, 2, 512], F32R, tag=f"pe{h}", bufs=2, name="p_exp"
                    )
                    nc.scalar.activation(
                        out=p_exp,
                        in_=ps_att,
                        func=mybir.ActivationFunctionType.Exp,
                        scale=SCALE,
                    )
                    for j in range(2):
                        jj = 2 * g + j - 4 * qc
                        if jj >= 0:  # diagonal block: causal 0/1 mask
                            off = 384 - 128 * jj
                            nc.vector.tensor_mul(
                                out=p_exp[:, j, :],
                                in0=p_exp[:, j, :],
                                in1=msk_sb[:, off : off + 512],
                            )
                    for j in range(2):
                        kb = 2 * g + j
                        nc.tensor.matmul(
                            ps_o[h][0 : HS + 1, :],
                            lhsT=_r(v_sb[h][:, kb, :], R_PV),
                            rhs=_r(p_exp[:, j, :], R_PV),
                            start=(kb == 0),
                            stop=(kb == nkb - 1),
                        )
            for h in range(HPC):
                r_sb = work.tile([1, 512], F32R, tag=f"r{h}", bufs=2, name="r_sb")
                nc.vector.reciprocal(out=r_sb, in_=ps_o[h][HS : HS + 1, :])
                ps_bc = pm.tile([HS, 512], F32, tag="mm", name="ps_bc")
                nc.tensor.matmul(
                    ps_bc,
                    lhsT=msk_sb[0:1, 384 : 384 + HS],
                    rhs=r_sb,
                    start=True,
                    stop=True,
                )
                rb_sb = work.tile([HS, 512], F32, tag=f"rb{h}", bufs=2, name="rb_sb")
                nc.any.tensor_copy(out=rb_sb, in_=ps_bc)
                nc.vector.tensor_mul(
                    out=attn_sb[h * HS : (h + 1) * HS, qc * 512 : (qc + 1) * 512],
                    in0=ps_o[h][0:HS, :],
                    in1=rb_sb,
                )

        # ---- phase 3: partial output projection -----------------------------
        for sc in range(NSC):
            out_t = work.tile([P, D], F32, tag="out", bufs=2, name="out_t")
            for dc in range(2):
                ps_p3 = pm.tile([P, 512], F32, tag="mm", name="ps_p3")
                nc.tensor.matmul(
                    ps_p3,
                    lhsT=_r(attn_sb[:, sc * P : (sc + 1) * P], R_P3),
                    rhs=_r(wp_sb[:, dc * 512 : (dc + 1) * 512], R_P3),
                    start=True,
                    stop=True,
                )
                nc.any.tensor_copy(out=out_t[:, dc * 512 : (dc + 1) * 512], in_=ps_p3)
            nc.sync.dma_start(out=out.ap()[sc * P : (sc + 1) * P, :], in_=out_t)

    nc.compile()
    return nc


_NC = None


def _get_nc():
    global _NC
    if _NC is None:
        _NC = _build()
    return _NC


def prepare_inputs(hidden_states, W_attn, b_attn, W_proj, b_proj):
    hs = np.asarray(hidden_states, dtype=np.float32)
    Wa = np.asarray(W_attn, dtype=np.float32)
    ba = np.asarray(b_attn, dtype=np.float32)
    Wp = np.asarray(W_proj, dtype=np.float32)

    hsT = np.ascontiguousarray(hs.T)
    pcol = np.arange(P)[:, None]
    ccol = np.arange(896)[None, :]
    msk = (pcol <= ccol - 384).astype(np.float32)

    in_maps = []
    for c in range(NCORES):
        q0 = c * CD
        wq = Wa[:, q0 : q0 + CD]
        wk = Wa[:, D + q0 : D + q0 + CD]
        wv = Wa[:, 2 * D + q0 : 2 * D + q0 + CD]
        bq = ba[q0 : q0 + CD]
        bk = ba[D + q0 : D + q0 + CD]
        bv = ba[2 * D + q0 : 2 * D + q0 + CD]
        in_maps.append(
            {
                "hsT": hsT,
                "w_qkv": np.ascontiguousarray(
                    np.concatenate([wq, wk, wv], axis=1), dtype=np.float32
                ),
                "b_qkv": np.ascontiguousarray(
                    np.stack([bq, bk, bv], axis=1), dtype=np.float32
                ),
                "w_p": np.ascontiguousarray(Wp[q0 : q0 + CD, :], dtype=np.float32),
                "msk": msk,
                "iden": np.eye(P, dtype=np.float32),
                "vones": np.ones((P, NSC), dtype=np.float32),
            }
        )
    return in_maps


def run(inputs, trace=False):
    """Build+run the sharded kernel. Returns (full_output, BassKernelResults)."""
    in_maps = prepare_inputs(**inputs)
    nc = _get_nc()
    res = run_bass_kernel_spmd(
        nc, in_maps, core_ids=list(range(NCORES)), trace=trace
    )
    acc = np.zeros((S, D), dtype=np.float32)
    for c in range(NCORES):
        acc += res.results[c]["out"]
    acc += np.asarray(inputs["b_proj"], dtype=np.float32)
    return acc, res


def kernel(**inputs):
    out, _ = run(inputs, trace=False)
    return out


# revision 13
# speedup vs baseline: 1.9199x; 1.0710x over previous
"""Causal self-attention (S=2048, D=1024, H=16) on 8 Trainium2 NeuronCores.

Sharding: tensor-parallel over heads. Core c owns heads 2c, 2c+1:
  - computes qT/kT/vT for its 128 qkv-columns from the full hidden_states
    (contraction layouts; vT is PE-transposed back to natural [s, j]),
  - runs causal attention for its 2 heads (attT = K.Q^T blocks, exp via
    ScalarE, denominators via a ones-column in the PV matmul, final
    normalization as a broadcast-denominator divide),
  - contracts its 128 head-dims against its W_proj row-slice,
  - outputs a partial [S, D] product; the host sums the 8 partials and
    adds b_proj.

Matmuls run in float32r (fp32 storage, single-pass PE = 4x the fp32 rate).
Phase 2 is software-pipelined (logits for group g overlap PV of group g-1)
to keep the PE dense and the HAM clock warm.
"""

import math
from contextlib import ExitStack

import numpy as np

import concourse.bacc as bacc
import concourse.mybir as mybir
import concourse.tile as tile
from concourse.bass_utils import run_bass_kernel_spmd

S, D, H = 2048, 1024, 16
HS = D // H  # 64 head size
P = 128
NCORES = 8
HPC = H // NCORES  # 2 heads per core
CD = HPC * HS  # 128 per-core head dims
KO = D // P  # 8 contraction tiles for the projections
NQC = S // 512  # 4 query chunks
NSC = S // P  # 16 sequence chunks of 128
SCALE = 1.0 / math.sqrt(S)

F32 = mybir.dt.float32
F32R = mybir.dt.float32r


def _build():
    nc = bacc.Bacc(
        "TRN2", target_bir_lowering=False, debug=False, num_devices=NCORES
    )

    hsT = nc.dram_tensor("hsT", [D, S], F32R, kind="ExternalInput")
    w_qkv = nc.dram_tensor("w_qkv", [D, 3 * P], F32R, kind="ExternalInput")
    b_qkv = nc.dram_tensor("b_qkv", [P, 3], F32, kind="ExternalInput")
    w_p = nc.dram_tensor("w_p", [CD, D], F32R, kind="ExternalInput")
    msk = nc.dram_tensor("msk", [P, 896], F32R, kind="ExternalInput")
    iden = nc.dram_tensor("iden", [P, P], F32R, kind="ExternalInput")
    vones = nc.dram_tensor("vones", [P, NSC], F32R, kind="ExternalInput")
    out = nc.dram_tensor("out", [S, D], F32, kind="ExternalOutput")

    with (
        tile.TileContext(nc) as tc,
        ExitStack() as ctx,
        nc.allow_low_precision(reason="float32r matmul pipeline"),
    ):
        const = ctx.enter_context(tc.tile_pool(name="const", bufs=1))
        work = ctx.enter_context(tc.tile_pool(name="work", bufs=2))
        pp = ctx.enter_context(tc.tile_pool(name="pp", bufs=1, space="PSUM"))

        def psA(name):  # generic 2-bank matmul target, 3 slots
            return pp.tile([P, 2, 512], F32, tag="A", bufs=3, name=name)

        # ---- constant loads -------------------------------------------------
        wqkv_sb = const.tile([P, KO, 3 * P], F32R, tag="wqkv", name="wqkv_sb")
        nc.sync.dma_start(
            out=wqkv_sb, in_=w_qkv.ap().rearrange("(o p) m -> p o m", p=P)
        )
        bqkv_sb = const.tile([P, 3], F32, tag="bqkv", name="bqkv_sb")
        nc.sync.dma_start(out=bqkv_sb, in_=b_qkv.ap())
        wp_sb = const.tile([P, D], F32R, tag="wp", name="wp_sb")
        nc.sync.dma_start(out=wp_sb, in_=w_p.ap())
        msk_sb = const.tile([P, 896], F32R, tag="msk", name="msk_sb")
        nc.sync.dma_start(out=msk_sb, in_=msk.ap())
        ident = const.tile([P, P], F32R, tag="ident", name="ident")
        nc.sync.dma_start(out=ident, in_=iden.ap())

        # hsT loaded in (o, n) chunks so phase-1 matmuls start immediately
        hsT_sb = const.tile([P, KO, S], F32R, tag="hsT", name="hsT_sb")
        for n in range(NQC):
            for o in range(KO):
                nc.sync.dma_start(
                    out=hsT_sb[:, o, n * 512 : (n + 1) * 512],
                    in_=hsT.ap()[o * P : (o + 1) * P, n * 512 : (n + 1) * 512],
                )

        qkT_sb = const.tile([P, 2, S], F32R, tag="qkT", name="qkT_sb")
        vT_sb = const.tile([P, S], F32R, tag="vT", name="vT_sb")
        v_sb = []
        for h in range(HPC):
            vt = const.tile([P, NSC, HS + 1], F32R, tag=f"v{h}", name=f"v{h}_sb")
            nc.sync.dma_start(out=vt[:, :, HS], in_=vones.ap())
            v_sb.append(vt)
        attn_sb = const.tile([P, S], F32R, tag="attn", name="attn_sb")

        # ---- phase 1: qT, kT, vT ([j, s] layout) + v transposes -------------
        for n in range(NQC):
            for m in range(3):
                ps_qkv = psA("ps_qkv")[:, 0, :]
                for o in range(KO):
                    nc.tensor.matmul(
                        ps_qkv,
                        lhsT=wqkv_sb[:, o, m * P : (m + 1) * P],
                        rhs=hsT_sb[:, o, n * 512 : (n + 1) * 512],
                        start=(o == 0),
                        stop=(o == KO - 1),
                    )
                dst = (
                    qkT_sb[:, m, n * 512 : (n + 1) * 512]
                    if m < 2
                    else vT_sb[:, n * 512 : (n + 1) * 512]
                )
                nc.vector.tensor_scalar_add(
                    out=dst, in0=ps_qkv, scalar1=bqkv_sb[:, m : m + 1]
                )
            # transpose this n-chunk of vT into natural v layout
            for sc in range(4 * n, 4 * n + 4):
                ps_t = psA("ps_t")[:, 0, 0:P].bitcast(F32R)
                nc.tensor.transpose(ps_t, vT_sb[:, sc * P : (sc + 1) * P], ident)
                for h in range(HPC):
                    nc.vector.tensor_copy(
                        out=v_sb[h][:, sc, 0:HS], in_=ps_t[:, h * HS : (h + 1) * HS]
                    )

        # ---- phase 2: causal attention, software-pipelined ------------------
        for qc in range(NQC):
            ps_o = [
                pp.tile([P, 512], F32, tag="O", bufs=2, name=f"ps_o{h}")
                for h in range(HPC)
            ]
            nkb = 4 * (qc + 1)  # 128-wide key blocks in the causal span
            ngrp = nkb // 2

            def emit_pv(pend, nkb=nkb, ps_o=ps_o):
                pes, kbs = pend
                for h in range(HPC):
                    for j, kb in enumerate(kbs):
                        nc.tensor.matmul(
                            ps_o[h][0 : HS + 1, :],
                            lhsT=v_sb[h][:, kb, :],
                            rhs=pes[h][:, j, :],
                            start=(kb == 0),
                            stop=(kb == nkb - 1),
                        )

            pending = None  # exp'd logits awaiting their PV matmuls
            for g in range(ngrp):
                kbs = [2 * g, 2 * g + 1]
                # logits for both heads, adjacent for row-group packing
                ps_att = [psA(f"ps_att{h}") for h in range(HPC)]
                for j, kb in enumerate(kbs):
                    for h in range(HPC):
                        nc.tensor.matmul(
                            ps_att[h][:, j, :],
                            lhsT=qkT_sb[h * HS : (h + 1) * HS, 1, kb * P : (kb + 1) * P],
                            rhs=qkT_sb[h * HS : (h + 1) * HS, 0, qc * 512 : (qc + 1) * 512],
                            start=True,
                            stop=True,
                        )
                if pending is not None:
                    emit_pv(pending)
                pes = []
                for h in range(HPC):
                    p_exp = work.tile(
                        [P, 2, 512], F32R, tag=f"pe{h}", bufs=3, name="p_exp"
                    )
                    nc.scalar.activation(
                        out=p_exp,
                        in_=ps_att[h],
                        func=mybir.ActivationFunctionType.Exp,
                        scale=SCALE,
                    )
                    for j, kb in enumerate(kbs):
                        jj = kb - 4 * qc
                        if jj >= 0:  # diagonal block: causal 0/1 mask
                            off = 384 - 128 * jj
                            nc.vector.tensor_mul(
                                out=p_exp[:, j, :],
                                in0=p_exp[:, j, :],
                                in1=msk_sb[:, off : off + 512],
                            )
                    pes.append(p_exp)
                pending = (pes, kbs)
            emit_pv(pending)

            # normalization: u = [out_unnorm; s], attn = u / broadcast(s)
            for h in range(HPC):
                u_sb = work.tile(
                    [HS + 1, 512], F32R, tag=f"u{h}", bufs=2, name="u_sb"
                )
                nc.vector.tensor_copy(out=u_sb, in_=ps_o[h][0 : HS + 1, :])
                rr = work.tile([1, 512], F32R, tag=f"rr{h}", bufs=2, name="rr")
                nc.vector.reciprocal(out=rr, in_=u_sb[HS : HS + 1, :])
                ps_bc = psA("ps_bc")[0:HS, 0, :]
                nc.tensor.matmul(
                    ps_bc,
                    lhsT=msk_sb[0:1, 384 : 384 + HS],
                    rhs=rr,
                    start=True,
                    stop=True,
                )
                nc.vector.tensor_mul(
                    out=attn_sb[h * HS : (h + 1) * HS, qc * 512 : (qc + 1) * 512],
                    in0=u_sb[0:HS, :],
                    in1=ps_bc,
                )

        # ---- phase 3: partial output projection -----------------------------
        for sc in range(NSC):
            out_t = work.tile([P, D], F32, tag="out", bufs=2, name="out_t")
            for dc in range(2):
                ps_p3 = psA("ps_p3")[:, 0, :]
                nc.tensor.matmul(
                    ps_p3,
                    lhsT=attn_sb[:, sc * P : (sc + 1) * P],
                    rhs=wp_sb[:, dc * 512 : (dc + 1) * 512],
                    start=True,
                    stop=True,
                )
                nc.vector.tensor_copy(
                    out=out_t[:, dc * 512 : (dc + 1) * 512], in_=ps_p3
                )
            nc.sync.dma_start(out=out.ap()[sc * P : (sc + 1) * P, :], in_=out_t)

    nc.compile()
    return nc


_NC = None


def _get_nc():
    global _NC
    if _NC is None:
        _NC = _build()
    return _NC


def prepare_inputs(hidden_states, W_attn, b_attn, W_proj, b_proj):
    hs = np.asarray(hidden_states, dtype=np.float32)
    Wa = np.asarray(W_attn, dtype=np.float32)
    ba = np.asarray(b_attn, dtype=np.float32)
    Wp = np.asarray(W_proj, dtype=np.float32)

    hsT = np.ascontiguousarray(hs.T)
    pcol = np.arange(P)[:, None]
    ccol = np.arange(896)[None, :]
    msk = (pcol <= ccol - 384).astype(np.float32)

    in_maps = []
    for c in range(NCORES):
        q0 = c * CD
        wq = Wa[:, q0 : q0 + CD]
        wk = Wa[:, D + q0 : D + q0 + CD]
        wv = Wa[:, 2 * D + q0 : 2 * D + q0 + CD]
        bq = ba[q0 : q0 + CD]
        bk = ba[D + q0 : D + q0 + CD]
        bv = ba[2 * D + q0 : 2 * D + q0 + CD]
        in_maps.append(
            {
                "hsT": hsT,
                "w_qkv": np.ascontiguousarray(
                    np.concatenate([wq, wk, wv], axis=1), dtype=np.float32
                ),
                "b_qkv": np.ascontiguousarray(
                    np.stack([bq, bk, bv], axis=1), dtype=np.float32
                ),
                "w_p": np.ascontiguousarray(Wp[q0 : q0 + CD, :], dtype=np.float32),
                "msk": msk,
                "iden": np.eye(P, dtype=np.float32),
                "vones": np.ones((P, NSC), dtype=np.float32),
            }
        )
    return in_maps


def run(inputs, trace=False):
    """Build+run the sharded kernel. Returns (full_output, BassKernelResults)."""
    in_maps = prepare_inputs(**inputs)
    nc = _get_nc()
    res = run_bass_kernel_spmd(
        nc, in_maps, core_ids=list(range(NCORES)), trace=trace
    )
    acc = np.zeros((S, D), dtype=np.float32)
    for c in range(NCORES):
        acc += res.results[c]["out"]
    acc += np.asarray(inputs["b_proj"], dtype=np.float32)
    return acc, res


def kernel(**inputs):
    out, _ = run(inputs, trace=False)
    return out


# revision 15
# speedup vs baseline: 2.2482x; 1.1710x over previous
"""Causal self-attention (S=2048, D=1024, H=16) on 8 Trainium2 NeuronCores.

Sharding: tensor-parallel over heads. Core c owns heads 2c, 2c+1:
  - computes qT/kT/vT for its 128 qkv-columns from the full hidden_states
    (contraction layouts; vT is PE-transposed back to natural [s, j]),
  - runs causal attention for its 2 heads (attT = K.Q^T blocks, exp via
    ScalarE, denominators via a ones-column in the PV matmul, final
    normalization as a broadcast-denominator divide),
  - contracts its 128 head-dims against its W_proj row-slice,
  - outputs a partial [S, D] product; the host sums the 8 partials and
    adds b_proj.

Matmuls run in float32r (fp32 storage, single-pass PE = 4x the fp32 rate).
Phase 2 is software-pipelined (logits for group g overlap PV of group g-1)
to keep the PE dense and the HAM clock warm.
"""

import math
from contextlib import ExitStack

import numpy as np

import concourse.bacc as bacc
import concourse.mybir as mybir
import concourse.tile as tile
from concourse.bass_utils import run_bass_kernel_spmd

S, D, H = 2048, 1024, 16
HS = D // H  # 64 head size
P = 128
NCORES = 8
HPC = H // NCORES  # 2 heads per core
CD = HPC * HS  # 128 per-core head dims
KO = D // P  # 8 contraction tiles for the projections
NQC = S // 512  # 4 query chunks
NSC = S // P  # 16 sequence chunks of 128
SCALE = 1.0 / math.sqrt(S)

F32 = mybir.dt.float32
F32R = mybir.dt.float32r


def _build():
    nc = bacc.Bacc(
        "TRN2", target_bir_lowering=False, debug=False, num_devices=NCORES
    )

    hsT = nc.dram_tensor("hsT", [D, S], F32R, kind="ExternalInput")
    w_qkv = nc.dram_tensor("w_qkv", [D, 3 * P], F32R, kind="ExternalInput")
    b_qkv = nc.dram_tensor("b_qkv", [P, 3], F32, kind="ExternalInput")
    w_p = nc.dram_tensor("w_p", [CD, D], F32R, kind="ExternalInput")
    msk = nc.dram_tensor("msk", [P, 896], F32R, kind="ExternalInput")
    iden = nc.dram_tensor("iden", [P, P], F32R, kind="ExternalInput")
    vones = nc.dram_tensor("vones", [P, NSC], F32R, kind="ExternalInput")
    out = nc.dram_tensor("out", [S, D], F32, kind="ExternalOutput")

    with (
        tile.TileContext(nc) as tc,
        ExitStack() as ctx,
        nc.allow_low_precision(reason="float32r matmul pipeline"),
    ):
        const = ctx.enter_context(tc.tile_pool(name="const", bufs=1))
        work = ctx.enter_context(tc.tile_pool(name="work", bufs=2))
        pp = ctx.enter_context(tc.tile_pool(name="pp", bufs=1, space="PSUM"))

        def psA(name):  # generic 2-bank matmul target, 3 slots
            return pp.tile([P, 2, 512], F32, tag="A", bufs=3, name=name)

        # ---- constant loads -------------------------------------------------
        wqkv_sb = const.tile([P, KO, 3 * P], F32R, tag="wqkv", name="wqkv_sb")
        nc.sync.dma_start(
            out=wqkv_sb, in_=w_qkv.ap().rearrange("(o p) m -> p o m", p=P)
        )
        bqkv_sb = const.tile([P, 3], F32, tag="bqkv", name="bqkv_sb")
        nc.sync.dma_start(out=bqkv_sb, in_=b_qkv.ap())
        wp_sb = const.tile([P, D], F32R, tag="wp", name="wp_sb")
        nc.sync.dma_start(out=wp_sb, in_=w_p.ap())
        msk_sb = const.tile([P, 896], F32R, tag="msk", name="msk_sb")
        nc.sync.dma_start(out=msk_sb, in_=msk.ap())
        ident = const.tile([P, P], F32R, tag="ident", name="ident")
        nc.sync.dma_start(out=ident, in_=iden.ap())

        # hsT loaded in (o, n) chunks so phase-1 matmuls start immediately
        hsT_sb = const.tile([P, KO, S], F32R, tag="hsT", name="hsT_sb")
        for n in range(NQC):
            for o in range(KO):
                nc.sync.dma_start(
                    out=hsT_sb[:, o, n * 512 : (n + 1) * 512],
                    in_=hsT.ap()[o * P : (o + 1) * P, n * 512 : (n + 1) * 512],
                )

        qkT_sb = const.tile([P, 2, S], F32R, tag="qkT", name="qkT_sb")
        vT_sb = const.tile([P, S], F32R, tag="vT", name="vT_sb")
        v_sb = []
        for h in range(HPC):
            vt = const.tile([P, NSC, HS + 1], F32R, tag=f"v{h}", name=f"v{h}_sb")
            nc.sync.dma_start(out=vt[:, :, HS], in_=vones.ap())
            v_sb.append(vt)
        sden_sb = const.tile([8, 512], F32R, tag="sden", name="sden_sb")
        u2_sb = [
            const.tile([P, 512], F32R, tag=f"u2_{qc}", name=f"u2_{qc}")
            for qc in range(NQC)
        ]
        rT_sb = const.tile([P, 32], F32, tag="rT", name="rT_sb")

        # ---- phase 1: qT, kT, vT ([j, s] layout) + v transposes -------------
        for n in range(NQC):
            for m in range(3):
                ps_qkv = psA("ps_qkv")[:, 0, :]
                for o in range(KO):
                    nc.tensor.matmul(
                        ps_qkv,
                        lhsT=wqkv_sb[:, o, m * P : (m + 1) * P],
                        rhs=hsT_sb[:, o, n * 512 : (n + 1) * 512],
                        start=(o == 0),
                        stop=(o == KO - 1),
                    )
                dst = (
                    qkT_sb[:, m, n * 512 : (n + 1) * 512]
                    if m < 2
                    else vT_sb[:, n * 512 : (n + 1) * 512]
                )
                nc.vector.tensor_scalar_add(
                    out=dst, in0=ps_qkv, scalar1=bqkv_sb[:, m : m + 1]
                )
            # transpose this n-chunk of vT into natural v layout
            for sc in range(4 * n, 4 * n + 4):
                ps_t = psA("ps_t")[:, 0, 0:P].bitcast(F32R)
                nc.tensor.transpose(ps_t, vT_sb[:, sc * P : (sc + 1) * P], ident)
                for h in range(HPC):
                    nc.scalar.copy(
                        out=v_sb[h][:, sc, 0:HS], in_=ps_t[:, h * HS : (h + 1) * HS]
                    )

        # ---- phase 2: causal attention, software-pipelined ------------------
        for qc in range(NQC):
            ps_o = [
                pp.tile([P, 512], F32, tag="O", bufs=2, name=f"ps_o{h}")
                for h in range(HPC)
            ]
            nkb = 4 * (qc + 1)  # 128-wide key blocks in the causal span
            ngrp = nkb // 2

            def emit_pv(pend, nkb=nkb, ps_o=ps_o):
                pes, kbs = pend
                for h in range(HPC):
                    for j, kb in enumerate(kbs):
                        nc.tensor.matmul(
                            ps_o[h][0 : HS + 1, :],
                            lhsT=v_sb[h][:, kb, :],
                            rhs=pes[h][:, j, :],
                            start=(kb == 0),
                            stop=(kb == nkb - 1),
                        )

            pending = None  # exp'd logits awaiting their PV matmuls
            for g in range(ngrp):
                kbs = [2 * g, 2 * g + 1]
                # logits for both heads, adjacent for row-group packing
                ps_att = [psA(f"ps_att{h}") for h in range(HPC)]
                for j, kb in enumerate(kbs):
                    for h in range(HPC):
                        nc.tensor.matmul(
                            ps_att[h][:, j, :],
                            lhsT=qkT_sb[h * HS : (h + 1) * HS, 1, kb * P : (kb + 1) * P],
                            rhs=qkT_sb[h * HS : (h + 1) * HS, 0, qc * 512 : (qc + 1) * 512],
                            start=True,
                            stop=True,
                        )
                if pending is not None:
                    emit_pv(pending)
                pes = []
                for h in range(HPC):
                    p_exp = work.tile(
                        [P, 2, 512], F32R, tag=f"pe{h}", bufs=3, name="p_exp"
                    )
                    nc.scalar.activation(
                        out=p_exp,
                        in_=ps_att[h],
                        func=mybir.ActivationFunctionType.Exp,
                        scale=SCALE,
                    )
                    for j, kb in enumerate(kbs):
                        jj = kb - 4 * qc
                        if jj >= 0:  # diagonal block: causal 0/1 mask
                            off = 384 - 128 * jj
                            nc.vector.tensor_mul(
                                out=p_exp[:, j, :],
                                in0=p_exp[:, j, :],
                                in1=msk_sb[:, off : off + 512],
                            )
                    pes.append(p_exp)
                pending = (pes, kbs)
            emit_pv(pending)

            # stash unnormalized head outputs + denominator rows; frees PSUM
            for h in range(HPC):
                nc.vector.tensor_copy(
                    out=u2_sb[qc][h * HS : (h + 1) * HS, :], in_=ps_o[h][0:HS, :]
                )
                r = 2 * qc + h
                dr = work.tile([1, 512], F32R, tag=f"dr{h}", bufs=2, name="dr")
                nc.vector.tensor_copy(out=dr, in_=ps_o[h][HS : HS + 1, :])
                nc.sync.dma_start(out=sden_sb[r : r + 1, :], in_=dr)

        # ---- denominators: transpose to [q-on-partitions], one reciprocal ---
        ps_dT = psA("ps_dT")
        for f in range(4):
            nc.tensor.transpose(
                ps_dT[:, 0, f * 8 : (f + 1) * 8].bitcast(F32R),
                sden_sb[:, f * P : (f + 1) * P],
                ident[0:8, 0:8],
            )
        nc.vector.reciprocal(out=rT_sb, in_=ps_dT[:, 0, 0:32])

        def rinv(sc, h):  # [128,1] per-partition 1/denominator for s-chunk sc
            return rT_sb[:, (sc % 4) * 8 + 2 * (sc // 4) + h : (sc % 4) * 8 + 2 * (sc // 4) + h + 1]

        # ---- phase 3: per-head projection, normalization fused ---------------
        for sc in range(NSC):
            qc = sc // 4
            f = sc % 4
            out_t = work.tile([P, 2, 512], F32, tag="out", bufs=2, name="out_t")
            tmp_t = work.tile([P, 2, 512], F32, tag="tmp3", bufs=2, name="tmp_t")
            slots = [psA("ps_p3a"), psA("ps_p3b")]
            for dc in range(2):
                for h in range(HPC):
                    nc.tensor.matmul(
                        slots[dc][:, h, :],
                        lhsT=u2_sb[qc][h * HS : (h + 1) * HS, f * P : (f + 1) * P],
                        rhs=wp_sb[h * HS : (h + 1) * HS, dc * 512 : (dc + 1) * 512],
                        start=True,
                        stop=True,
                    )
            for dc in range(2):
                nc.scalar.mul(
                    out=tmp_t[:, dc, :], in_=slots[dc][:, 0, :], mul=rinv(sc, 0)
                )
                nc.vector.scalar_tensor_tensor(
                    out=out_t[:, dc, :],
                    in0=slots[dc][:, 1, :],
                    scalar=rinv(sc, 1),
                    in1=tmp_t[:, dc, :],
                    op0=mybir.AluOpType.mult,
                    op1=mybir.AluOpType.add,
                )
            nc.sync.dma_start(
                out=out.ap()[sc * P : (sc + 1) * P, :],
                in_=out_t.rearrange("p a b -> p (a b)"),
            )

    nc.compile()
    return nc


_NC = None


def _get_nc():
    global _NC
    if _NC is None:
        _NC = _build()
    return _NC


def prepare_inputs(hidden_states, W_attn, b_attn, W_proj, b_proj):
    hs = np.asarray(hidden_states, dtype=np.float32)
    Wa = np.asarray(W_attn, dtype=np.float32)
    ba = np.asarray(b_attn, dtype=np.float32)
    Wp = np.asarray(W_proj, dtype=np.float32)

    hsT = np.ascontiguousarray(hs.T)
    pcol = np.arange(P)[:, None]
    ccol = np.arange(896)[None, :]
    msk = (pcol <= ccol - 384).astype(np.float32)

    in_maps = []
    for c in range(NCORES):
        q0 = c * CD
        wq = Wa[:, q0 : q0 + CD]
        wk = Wa[:, D + q0 : D + q0 + CD]
        wv = Wa[:, 2 * D + q0 : 2 * D + q0 + CD]
        bq = ba[q0 : q0 + CD]
        bk = ba[D + q0 : D + q0 + CD]
        bv = ba[2 * D + q0 : 2 * D + q0 + CD]
        in_maps.append(
            {
                "hsT": hsT,
                "w_qkv": np.ascontiguousarray(
                    np.concatenate([wq, wk, wv], axis=1), dtype=np.float32
                ),
                "b_qkv": np.ascontiguousarray(
                    np.stack([bq, bk, bv], axis=1), dtype=np.float32
                ),
                "w_p": np.ascontiguousarray(Wp[q0 : q0 + CD, :], dtype=np.float32),
                "msk": msk,
                "iden": np.eye(P, dtype=np.float32),
                "vones": np.ones((P, NSC), dtype=np.float32),
            }
        )
    return in_maps


def run(inputs, trace=False):
    """Build+run the sharded kernel. Returns (full_output, BassKernelResults)."""
    in_maps = prepare_inputs(**inputs)
    nc = _get_nc()
    res = run_bass_kernel_spmd(
        nc, in_maps, core_ids=list(range(NCORES)), trace=trace
    )
    acc = np.zeros((S, D), dtype=np.float32)
    for c in range(NCORES):
        acc += res.results[c]["out"]
    acc += np.asarray(inputs["b_proj"], dtype=np.float32)
    return acc, res


def kernel(**inputs):
    out, _ = run(inputs, trace=False)
    return out


# revision 16
# speedup vs baseline: 2.3678x; 1.0532x over previous
"""Causal self-attention (S=2048, D=1024, H=16) on 8 Trainium2 NeuronCores.

Sharding: tensor-parallel over heads. Core c owns heads 2c, 2c+1:
  - computes qT/kT/vT for its 128 qkv-columns from the full hidden_states
    (contraction layouts; vT is PE-transposed back to natural [s, j]),
  - runs causal attention for its 2 heads (attT = K.Q^T blocks, exp via
    ScalarE, denominators via a ones-column in the PV matmul),
  - projects each head against its W_proj row-slice and fuses the softmax
    normalization into the projection epilogue (per-partition 1/den scales,
    denominators PE-transposed so one wide reciprocal covers all of them),
  - outputs a partial [S, D] product; the host sums the 8 partials and
    adds b_proj.

The bulk matmuls run in bf16 (1 cycle/row); the unnormalized attention
outputs, denominators and the final projection stay in float32r (fp32
storage, single-pass PE). Phase 2 is software-pipelined (logits of group g
overlap the PV matmuls of group g-1) to keep the PE dense and the HAM
clock warm; dummy matmuls paced by the input DMA keep the clock warm
during the initial load.
"""

import math
from contextlib import ExitStack

import numpy as np

import concourse.bacc as bacc
import concourse.mybir as mybir
import concourse.tile as tile
from concourse.bass_utils import run_bass_kernel_spmd

S, D, H = 2048, 1024, 16
HS = D // H  # 64 head size
P = 128
NCORES = 8
HPC = H // NCORES  # 2 heads per core
CD = HPC * HS  # 128 per-core head dims
KO = D // P  # 8 contraction tiles for the projections
NQC = S // 512  # 4 query chunks
NSC = S // P  # 16 sequence chunks of 128
SCALE = 1.0 / math.sqrt(S)

F32 = mybir.dt.float32
F32R = mybir.dt.float32r
BF16 = mybir.dt.bfloat16

try:
    import ml_dtypes

    NP_BF16 = ml_dtypes.bfloat16
except ImportError:  # pragma: no cover
    NP_BF16 = None


def _build():
    nc = bacc.Bacc(
        "TRN2", target_bir_lowering=False, debug=False, num_devices=NCORES
    )

    hsT = nc.dram_tensor("hsT", [D, S], BF16, kind="ExternalInput")
    w_qkv = nc.dram_tensor("w_qkv", [D, 3 * P], BF16, kind="ExternalInput")
    b_qkv = nc.dram_tensor("b_qkv", [P, 3], F32, kind="ExternalInput")
    w_p = nc.dram_tensor("w_p", [CD, D], F32R, kind="ExternalInput")
    msk = nc.dram_tensor("msk", [P, 896], BF16, kind="ExternalInput")
    iden_b = nc.dram_tensor("iden_b", [P, P], BF16, kind="ExternalInput")
    iden_r = nc.dram_tensor("iden_r", [P, P], F32R, kind="ExternalInput")
    vones = nc.dram_tensor("vones", [P, NSC], BF16, kind="ExternalInput")
    out = nc.dram_tensor("out", [S, D], F32, kind="ExternalOutput")

    with (
        tile.TileContext(nc) as tc,
        ExitStack() as ctx,
        nc.allow_low_precision(reason="bf16/float32r matmul pipeline"),
    ):
        const = ctx.enter_context(tc.tile_pool(name="const", bufs=1))
        work = ctx.enter_context(tc.tile_pool(name="work", bufs=2))
        pp = ctx.enter_context(tc.tile_pool(name="pp", bufs=1, space="PSUM"))

        def psA(name):  # generic 2-bank matmul target, 3 slots
            return pp.tile([P, 2, 512], F32, tag="A", bufs=3, name=name)

        # ---- loads: identity first (pre-warm), then per-o weight+hsT chunks
        identb = const.tile([P, P], BF16, tag="identb", name="identb")
        nc.sync.dma_start(out=identb, in_=iden_b.ap())
        identr = const.tile([P, P], F32R, tag="identr", name="identr")
        nc.sync.dma_start(out=identr, in_=iden_r.ap())
        bqkv_sb = const.tile([P, 3], F32, tag="bqkv", name="bqkv_sb")
        nc.sync.dma_start(out=bqkv_sb, in_=b_qkv.ap())

        hsT_sb = const.tile([P, KO, S], BF16, tag="hsT", name="hsT_sb")
        wqkv_sb = const.tile([P, KO, 3 * P], BF16, tag="wqkv", name="wqkv_sb")
        for o in range(KO):
            nc.sync.dma_start(
                out=wqkv_sb[:, o, :], in_=w_qkv.ap()[o * P : (o + 1) * P, :]
            )
            nc.sync.dma_start(
                out=hsT_sb[:, o, 0:512], in_=hsT.ap()[o * P : (o + 1) * P, 0:512]
            )
        wp_sb = const.tile([P, D], F32R, tag="wp", name="wp_sb")
        nc.sync.dma_start(out=wp_sb, in_=w_p.ap())
        msk_sb = const.tile([P, 896], BF16, tag="msk", name="msk_sb")
        nc.sync.dma_start(out=msk_sb, in_=msk.ap())
        v_sb = []
        for h in range(HPC):
            vt = const.tile([P, NSC, HS + 1], BF16, tag=f"v{h}", name=f"v{h}_sb")
            nc.sync.dma_start(out=vt[:, :, HS], in_=vones.ap())
            v_sb.append(vt)
        for n in range(1, NQC):
            for o in range(KO):
                nc.sync.dma_start(
                    out=hsT_sb[:, o, n * 512 : (n + 1) * 512],
                    in_=hsT.ap()[o * P : (o + 1) * P, n * 512 : (n + 1) * 512],
                )

        qkT_sb = const.tile([P, 2, S], BF16, tag="qkT", name="qkT_sb")
        vT_sb = const.tile([P, S], BF16, tag="vT", name="vT_sb")
        sden_sb = const.tile([8, 512], F32R, tag="sden", name="sden_sb")
        u2_sb = [
            const.tile([P, 512], F32R, tag=f"u2_{qc}", name=f"u2_{qc}")
            for qc in range(NQC)
        ]
        rT_sb = const.tile([P, 32], F32, tag="rT", name="rT_sb")

        # ---- pre-warm the PE clock while the DMAs stream ---------------------
        # each burst consumes a freshly-arrived hsT chunk so the bursts are
        # spread across the load instead of back-to-back at t=0
        ps_w = psA("ps_w")
        for o in range(KO):
            for rep in range(6):
                nc.tensor.matmul(
                    ps_w[:, 0, :],
                    lhsT=identb,
                    rhs=hsT_sb[:, o, 0:512],
                    start=True,
                    stop=True,
                )

        # ---- phase 1: qT, kT, vT ([j, s] layout) + v transposes -------------
        for n in range(NQC):
            for m in range(3):
                ps_qkv = psA("ps_qkv")[:, 0, :]
                for o in range(KO):
                    nc.tensor.matmul(
                        ps_qkv,
                        lhsT=wqkv_sb[:, o, m * P : (m + 1) * P],
                        rhs=hsT_sb[:, o, n * 512 : (n + 1) * 512],
                        start=(o == 0),
                        stop=(o == KO - 1),
                    )
                dst = (
                    qkT_sb[:, m, n * 512 : (n + 1) * 512]
                    if m < 2
                    else vT_sb[:, n * 512 : (n + 1) * 512]
                )
                nc.vector.tensor_scalar_add(
                    out=dst, in0=ps_qkv, scalar1=bqkv_sb[:, m : m + 1]
                )
            # transpose this n-chunk of vT into natural v layout
            for sc in range(4 * n, 4 * n + 4):
                ps_t = pp.tile([P, P], BF16, tag="A", bufs=3, name="ps_t")
                nc.tensor.transpose(ps_t, vT_sb[:, sc * P : (sc + 1) * P], identb)
                for h in range(HPC):
                    nc.vector.tensor_copy(
                        out=v_sb[h][:, sc, 0:HS], in_=ps_t[:, h * HS : (h + 1) * HS]
                    )

        # ---- phase 2: causal attention, software-pipelined ------------------
        for qc in range(NQC):
            ps_o = [
                pp.tile([P, 512], F32, tag="O", bufs=2, name=f"ps_o{h}")
                for h in range(HPC)
            ]
            nkb = 4 * (qc + 1)  # 128-wide key blocks in the causal span
            ngrp = nkb // 2

            def emit_pv(pend, nkb=nkb, ps_o=ps_o):
                pes, kbs = pend
                for h in range(HPC):
                    for j, kb in enumerate(kbs):
                        nc.tensor.matmul(
                            ps_o[h][0 : HS + 1, :],
                            lhsT=v_sb[h][:, kb, :],
                            rhs=pes[h][:, j, :],
                            start=(kb == 0),
                            stop=(kb == nkb - 1),
                        )

            pending = None  # exp'd logits awaiting their PV matmuls
            for g in range(ngrp):
                kbs = [2 * g, 2 * g + 1]
                # logits for both heads, adjacent for row-group packing
                ps_att = [psA(f"ps_att{h}") for h in range(HPC)]
                for j, kb in enumerate(kbs):
                    for h in range(HPC):
                        nc.tensor.matmul(
                            ps_att[h][:, j, :],
                            lhsT=qkT_sb[h * HS : (h + 1) * HS, 1, kb * P : (kb + 1) * P],
                            rhs=qkT_sb[h * HS : (h + 1) * HS, 0, qc * 512 : (qc + 1) * 512],
                            start=True,
                            stop=True,
                        )
                if pending is not None:
                    emit_pv(pending)
                pes = []
                for h in range(HPC):
                    p_exp = work.tile(
                        [P, 2, 512], BF16, tag=f"pe{h}", bufs=3, name="p_exp"
                    )
                    nc.scalar.activation(
                        out=p_exp,
                        in_=ps_att[h],
                        func=mybir.ActivationFunctionType.Exp,
                        scale=SCALE,
                    )
                    for j, kb in enumerate(kbs):
                        jj = kb - 4 * qc
                        if jj >= 0:  # diagonal block: causal 0/1 mask
                            off = 384 - 128 * jj
                            nc.vector.tensor_mul(
                                out=p_exp[:, j, :],
                                in0=p_exp[:, j, :],
                                in1=msk_sb[:, off : off + 512],
                            )
                    pes.append(p_exp)
                pending = (pes, kbs)
            emit_pv(pending)

            # stash unnormalized head outputs + denominator rows; frees PSUM
            for h in range(HPC):
                nc.vector.tensor_copy(
                    out=u2_sb[qc][h * HS : (h + 1) * HS, :], in_=ps_o[h][0:HS, :]
                )
                r = 2 * qc + h
                dr = work.tile([1, 512], F32R, tag=f"dr{h}", bufs=2, name="dr")
                nc.vector.tensor_copy(out=dr, in_=ps_o[h][HS : HS + 1, :])
                nc.sync.dma_start(out=sden_sb[r : r + 1, :], in_=dr)

        # ---- denominators: transpose to [q-on-partitions], one reciprocal ---
        ps_dT = psA("ps_dT")
        for f in range(4):
            nc.tensor.transpose(
                ps_dT[:, 0, f * 8 : (f + 1) * 8].bitcast(F32R),
                sden_sb[:, f * P : (f + 1) * P],
                identr[0:8, 0:8],
            )
        nc.vector.reciprocal(out=rT_sb, in_=ps_dT[:, 0, 0:32])

        def rinv(sc, h):  # [128,1] per-partition 1/denominator for s-chunk sc
            i = (sc % 4) * 8 + 2 * (sc // 4) + h
            return rT_sb[:, i : i + 1]

        # ---- phase 3: per-head projection, normalization fused ---------------
        for sc in range(NSC):
            qc = sc // 4
            f = sc % 4
            out_t = work.tile([P, 2, 512], F32, tag="out", bufs=2, name="out_t")
            tmp_t = work.tile([P, 2, 512], F32, tag="tmp3", bufs=2, name="tmp_t")
            slots = [psA("ps_p3a"), psA("ps_p3b")]  # slots[h]: banks = dc
            for dc in range(2):
                for h in range(HPC):
                    nc.tensor.matmul(
                        slots[h][:, dc, :],
                        lhsT=u2_sb[qc][h * HS : (h + 1) * HS, f * P : (f + 1) * P],
                        rhs=wp_sb[h * HS : (h + 1) * HS, dc * 512 : (dc + 1) * 512],
                        start=True,
                        stop=True,
                    )
            nc.scalar.mul(out=tmp_t, in_=slots[0], mul=rinv(sc, 0))
            nc.vector.scalar_tensor_tensor(
                out=out_t,
                in0=slots[1],
                scalar=rinv(sc, 1),
                in1=tmp_t,
                op0=mybir.AluOpType.mult,
                op1=mybir.AluOpType.add,
            )
            nc.sync.dma_start(
                out=out.ap()[sc * P : (sc + 1) * P, :],
                in_=out_t.rearrange("p a b -> p (a b)"),
            )

    nc.compile()
    return nc


_NC = None


def _get_nc():
    global _NC
    if _NC is None:
        _NC = _build()
    return _NC


def prepare_inputs(hidden_states, W_attn, b_attn, W_proj, b_proj):
    hs = np.asarray(hidden_states, dtype=np.float32)
    Wa = np.asarray(W_attn, dtype=np.float32)
    ba = np.asarray(b_attn, dtype=np.float32)
    Wp = np.asarray(W_proj, dtype=np.float32)

    hsT = np.ascontiguousarray(hs.T).astype(NP_BF16)
    pcol = np.arange(P)[:, None]
    ccol = np.arange(896)[None, :]
    msk = (pcol <= ccol - 384).astype(NP_BF16)

    in_maps = []
    for c in range(NCORES):
        q0 = c * CD
        wq = Wa[:, q0 : q0 + CD]
        wk = Wa[:, D + q0 : D + q0 + CD]
        wv = Wa[:, 2 * D + q0 : 2 * D + q0 + CD]
        bq = ba[q0 : q0 + CD]
        bk = ba[D + q0 : D + q0 + CD]
        bv = ba[2 * D + q0 : 2 * D + q0 + CD]
        in_maps.append(
            {
                "hsT": hsT,
                "w_qkv": np.ascontiguousarray(
                    np.concatenate([wq, wk, wv], axis=1)
                ).astype(NP_BF16),
                "b_qkv": np.ascontiguousarray(np.stack([bq, bk, bv], axis=1)).astype(
                    np.float32
                ),
                "w_p": np.ascontiguousarray(Wp[q0 : q0 + CD, :], dtype=np.float32),
                "msk": msk,
                "iden_b": np.eye(P).astype(NP_BF16),
                "iden_r": np.eye(P, dtype=np.float32),
                "vones": np.ones((P, NSC)).astype(NP_BF16),
            }
        )
    return in_maps


def run(inputs, trace=False):
    """Build+run the sharded kernel. Returns (full_output, BassKernelResults)."""
    in_maps = prepare_inputs(**inputs)
    nc = _get_nc()
    res = run_bass_kernel_spmd(
        nc, in_maps, core_ids=list(range(NCORES)), trace=trace
    )
    acc = np.zeros((S, D), dtype=np.float32)
    for c in range(NCORES):
        acc += res.results[c]["out"]
    acc += np.asarray(inputs["b_proj"], dtype=np.float32)
    return acc, res


def kernel(**inputs):
    out, _ = run(inputs, trace=False)
    return out


# revision 17
# speedup vs baseline: 2.4304x; 1.0265x over previous
"""Causal self-attention (S=2048, D=1024, H=16) on 8 Trainium2 NeuronCores.

Sharding: tensor-parallel over heads. Core c owns heads 2c, 2c+1:
  - computes qT/kT/vT for its 128 qkv-columns from the full hidden_states
    (contraction layouts; vT is PE-transposed back to natural [s, j]),
  - runs causal attention for its 2 heads (attT = K.Q^T blocks, exp via
    ScalarE, denominators via a ones-column in the PV matmul),
  - projects each head against its W_proj row-slice and fuses the softmax
    normalization into the projection epilogue (per-partition 1/den scales,
    denominators PE-transposed so one wide reciprocal covers all of them),
  - outputs a partial [S, D] product; the host sums the 8 partials and
    adds b_proj.

The bulk matmuls run in bf16 (1 cycle/row); the unnormalized attention
outputs, denominators and the final projection stay in float32r (fp32
storage, single-pass PE). Phase 2 is software-pipelined (logits of group g
overlap the PV matmuls of group g-1) to keep the PE dense and the HAM
clock warm; dummy matmuls paced by the input DMA keep the clock warm
during the initial load.
"""

import math
from contextlib import ExitStack

import numpy as np

import concourse.bacc as bacc
import concourse.mybir as mybir
import concourse.tile as tile
from concourse.bass_utils import run_bass_kernel_spmd

S, D, H = 2048, 1024, 16
HS = D // H  # 64 head size
P = 128
NCORES = 8
HPC = H // NCORES  # 2 heads per core
CD = HPC * HS  # 128 per-core head dims
KO = D // P  # 8 contraction tiles for the projections
NQC = S // 512  # 4 query chunks
NSC = S // P  # 16 sequence chunks of 128
SCALE = 1.0 / math.sqrt(S)

F32 = mybir.dt.float32
F32R = mybir.dt.float32r
BF16 = mybir.dt.bfloat16

try:
    import ml_dtypes

    NP_BF16 = ml_dtypes.bfloat16
except ImportError:  # pragma: no cover
    NP_BF16 = None


def _build():
    nc = bacc.Bacc(
        "TRN2", target_bir_lowering=False, debug=False, num_devices=NCORES
    )

    hsT = nc.dram_tensor("hsT", [D, S], BF16, kind="ExternalInput")
    w_qkv = nc.dram_tensor("w_qkv", [D, 3 * P], BF16, kind="ExternalInput")
    b_qkv = nc.dram_tensor("b_qkv", [P, 3], F32, kind="ExternalInput")
    w_p = nc.dram_tensor("w_p", [CD, D], F32R, kind="ExternalInput")
    msk = nc.dram_tensor("msk", [P, 896], BF16, kind="ExternalInput")
    iden_b = nc.dram_tensor("iden_b", [P, P], BF16, kind="ExternalInput")
    iden_r = nc.dram_tensor("iden_r", [P, P], F32R, kind="ExternalInput")
    vones = nc.dram_tensor("vones", [P, NSC], BF16, kind="ExternalInput")
    out = nc.dram_tensor("out", [S, D], F32, kind="ExternalOutput")

    with (
        tile.TileContext(nc) as tc,
        ExitStack() as ctx,
        nc.allow_low_precision(reason="bf16/float32r matmul pipeline"),
    ):
        const = ctx.enter_context(tc.tile_pool(name="const", bufs=1))
        work = ctx.enter_context(tc.tile_pool(name="work", bufs=2))
        pp = ctx.enter_context(tc.tile_pool(name="pp", bufs=1, space="PSUM"))

        def psA(name):  # generic 2-bank matmul target, 3 slots
            return pp.tile([P, 2, 512], F32, tag="A", bufs=3, name=name)

        # ---- loads: identity first (pre-warm), then per-o weight+hsT chunks
        identb = const.tile([P, P], BF16, tag="identb", name="identb")
        nc.sync.dma_start(out=identb, in_=iden_b.ap())
        identr = const.tile([P, P], F32R, tag="identr", name="identr")
        nc.sync.dma_start(out=identr, in_=iden_r.ap())
        bqkv_sb = const.tile([P, 3], F32, tag="bqkv", name="bqkv_sb")
        nc.sync.dma_start(out=bqkv_sb, in_=b_qkv.ap())

        hsT_sb = const.tile([P, KO, S], BF16, tag="hsT", name="hsT_sb")
        wqkv_sb = const.tile([P, KO, 3 * P], BF16, tag="wqkv", name="wqkv_sb")
        for o in range(KO):
            nc.sync.dma_start(
                out=wqkv_sb[:, o, :], in_=w_qkv.ap()[o * P : (o + 1) * P, :]
            )
            nc.sync.dma_start(
                out=hsT_sb[:, o, 0:512], in_=hsT.ap()[o * P : (o + 1) * P, 0:512]
            )
        wp_sb = const.tile([P, D], F32R, tag="wp", name="wp_sb")
        nc.sync.dma_start(out=wp_sb, in_=w_p.ap())
        msk_sb = const.tile([P, 896], BF16, tag="msk", name="msk_sb")
        nc.sync.dma_start(out=msk_sb, in_=msk.ap())
        v_sb = []
        for h in range(HPC):
            vt = const.tile([P, NSC, HS + 1], BF16, tag=f"v{h}", name=f"v{h}_sb")
            nc.sync.dma_start(out=vt[:, :, HS], in_=vones.ap())
            v_sb.append(vt)
        for n in range(1, NQC):
            for o in range(KO):
                nc.sync.dma_start(
                    out=hsT_sb[:, o, n * 512 : (n + 1) * 512],
                    in_=hsT.ap()[o * P : (o + 1) * P, n * 512 : (n + 1) * 512],
                )

        qkT_sb = const.tile([P, 2, S], BF16, tag="qkT", name="qkT_sb")
        vT_sb = const.tile([P, S], BF16, tag="vT", name="vT_sb")
        sden_sb = const.tile([8, 512], F32R, tag="sden", name="sden_sb")
        u2_sb = [
            const.tile([P, 512], F32R, tag=f"u2_{qc}", name=f"u2_{qc}")
            for qc in range(NQC)
        ]
        rT_sb = const.tile([P, 32], F32, tag="rT", name="rT_sb")

        # ---- pre-warm the PE clock while the DMAs stream ---------------------
        # each burst consumes a freshly-arrived hsT chunk so the bursts are
        # spread across the load instead of back-to-back at t=0
        ps_w = psA("ps_w")
        for o in range(KO):
            for rep in range(6):
                nc.tensor.matmul(
                    ps_w[:, 0, :],
                    lhsT=identb,
                    rhs=hsT_sb[:, o, 0:512],
                    start=True,
                    stop=True,
                )

        # ---- phase 1: qT, kT, vT ([j, s] layout) + v transposes -------------
        for n in range(NQC):
            for m in range(3):
                ps_qkv = psA("ps_qkv")[:, 0, :]
                for o in range(KO):
                    nc.tensor.matmul(
                        ps_qkv,
                        lhsT=wqkv_sb[:, o, m * P : (m + 1) * P],
                        rhs=hsT_sb[:, o, n * 512 : (n + 1) * 512],
                        start=(o == 0),
                        stop=(o == KO - 1),
                    )
                dst = (
                    qkT_sb[:, m, n * 512 : (n + 1) * 512]
                    if m < 2
                    else vT_sb[:, n * 512 : (n + 1) * 512]
                )
                nc.vector.tensor_scalar_add(
                    out=dst, in0=ps_qkv, scalar1=bqkv_sb[:, m : m + 1]
                )
            # transpose this n-chunk of vT into natural v layout
            for sc in range(4 * n, 4 * n + 4):
                ps_t = pp.tile([P, P], BF16, tag="A", bufs=3, name="ps_t")
                nc.tensor.transpose(ps_t, vT_sb[:, sc * P : (sc + 1) * P], identb)
                for h in range(HPC):
                    nc.vector.tensor_copy(
                        out=v_sb[h][:, sc, 0:HS], in_=ps_t[:, h * HS : (h + 1) * HS]
                    )

        # ---- phase 2: causal attention, software-pipelined ------------------
        for qc in range(NQC):
            ps_o = [
                pp.tile([P, 512], F32, tag="O", bufs=2, name=f"ps_o{h}")
                for h in range(HPC)
            ]
            nkb = 4 * (qc + 1)  # 128-wide key blocks in the causal span
            ngrp = nkb // 2

            def emit_pv(pend, nkb=nkb, ps_o=ps_o):
                pes, kbs, f0 = pend
                for h in range(HPC):
                    for j, kb in enumerate(kbs):
                        nc.tensor.matmul(
                            ps_o[h][0 : HS + 1, f0:512],
                            lhsT=v_sb[h][:, kb, :],
                            rhs=pes[h][:, j, f0:512],
                            start=(kb == 0),
                            stop=(kb == nkb - 1),
                        )

            pending = None  # exp'd logits awaiting their PV matmuls
            for g in range(ngrp):
                kbs = [2 * g, 2 * g + 1]
                # last group covers only the causal upper half of the q range
                f0 = 256 if g == ngrp - 1 else 0
                # logits for both heads, adjacent for row-group packing
                ps_att = [psA(f"ps_att{h}") for h in range(HPC)]
                for j, kb in enumerate(kbs):
                    for h in range(HPC):
                        nc.tensor.matmul(
                            ps_att[h][:, j, f0:512],
                            lhsT=qkT_sb[h * HS : (h + 1) * HS, 1, kb * P : (kb + 1) * P],
                            rhs=qkT_sb[h * HS : (h + 1) * HS, 0, qc * 512 + f0 : (qc + 1) * 512],
                            start=True,
                            stop=True,
                        )
                if pending is not None:
                    emit_pv(pending)
                pes = []
                for h in range(HPC):
                    p_exp = work.tile(
                        [P, 2, 512], BF16, tag=f"pe{h}", bufs=4, name="p_exp"
                    )
                    nc.scalar.activation(
                        out=p_exp[:, :, f0:512],
                        in_=ps_att[h][:, :, f0:512],
                        func=mybir.ActivationFunctionType.Exp,
                        scale=SCALE,
                    )
                    for j, kb in enumerate(kbs):
                        jj = kb - 4 * qc
                        if jj >= 0:  # diagonal block: causal 0/1 mask
                            off = 384 - 128 * jj
                            nc.vector.tensor_mul(
                                out=p_exp[:, j, f0:512],
                                in0=p_exp[:, j, f0:512],
                                in1=msk_sb[:, off + f0 : off + 512],
                            )
                    pes.append(p_exp)
                pending = (pes, kbs, f0)
            emit_pv(pending)

            # stash unnormalized head outputs + denominator rows; frees PSUM
            for h in range(HPC):
                nc.vector.tensor_copy(
                    out=u2_sb[qc][h * HS : (h + 1) * HS, :], in_=ps_o[h][0:HS, :]
                )
                r = 2 * qc + h
                dr = work.tile([1, 512], F32R, tag=f"dr{h}", bufs=2, name="dr")
                nc.vector.tensor_copy(out=dr, in_=ps_o[h][HS : HS + 1, :])
                nc.sync.dma_start(out=sden_sb[r : r + 1, :], in_=dr)

        def rinv(sc, h):  # [128,1] per-partition 1/denominator for s-chunk sc
            i = (sc % 4) * 8 + 2 * (sc // 4) + h
            return rT_sb[:, i : i + 1]

        # ---- phase 3: per-head projection, normalization fused ---------------
        # software-pipelined: matmuls run 2 chunks ahead of the epilogues; the
        # denominator transposes + single wide reciprocal slot in behind the
        # first chunks' matmuls
        def emit_p3_mm(sc):
            qc = sc // 4
            f = sc % 4
            slots = [psA("ps_p3a"), psA("ps_p3b")]  # slots[h]: banks = dc
            for dc in range(2):
                for h in range(HPC):
                    nc.tensor.matmul(
                        slots[h][:, dc, :],
                        lhsT=u2_sb[qc][h * HS : (h + 1) * HS, f * P : (f + 1) * P],
                        rhs=wp_sb[h * HS : (h + 1) * HS, dc * 512 : (dc + 1) * 512],
                        start=True,
                        stop=True,
                    )
            return slots

        def emit_p3_epi(sc, slots):
            out_t = work.tile([P, 2, 512], F32, tag="out", bufs=2, name="out_t")
            tmp_t = work.tile([P, 2, 512], F32, tag="tmp3", bufs=2, name="tmp_t")
            nc.scalar.mul(out=tmp_t, in_=slots[0], mul=rinv(sc, 0))
            nc.vector.scalar_tensor_tensor(
                out=out_t,
                in0=slots[1],
                scalar=rinv(sc, 1),
                in1=tmp_t,
                op0=mybir.AluOpType.mult,
                op1=mybir.AluOpType.add,
            )
            nc.sync.dma_start(
                out=out.ap()[sc * P : (sc + 1) * P, :],
                in_=out_t.rearrange("p a b -> p (a b)"),
            )

        p3q = []
        for sc in range(NSC):
            p3q.append((sc, emit_p3_mm(sc)))
            if sc == 0:
                # denominators: transpose to [q-on-partitions], one reciprocal
                ps_dT = psA("ps_dT")
                for f in range(4):
                    nc.tensor.transpose(
                        ps_dT[:, 0, f * 8 : (f + 1) * 8].bitcast(F32R),
                        sden_sb[:, f * P : (f + 1) * P],
                        identr[0:8, 0:8],
                    )
                nc.vector.reciprocal(out=rT_sb, in_=ps_dT[:, 0, 0:32])
            if len(p3q) > 2:
                emit_p3_epi(*p3q.pop(0))
        for item in p3q:
            emit_p3_epi(*item)

    nc.compile()
    return nc


_NC = None


def _get_nc():
    global _NC
    if _NC is None:
        _NC = _build()
    return _NC


def prepare_inputs(hidden_states, W_attn, b_attn, W_proj, b_proj):
    hs = np.asarray(hidden_states, dtype=np.float32)
    Wa = np.asarray(W_attn, dtype=np.float32)
    ba = np.asarray(b_attn, dtype=np.float32)
    Wp = np.asarray(W_proj, dtype=np.float32)

    hsT = np.ascontiguousarray(hs.T).astype(NP_BF16)
    pcol = np.arange(P)[:, None]
    ccol = np.arange(896)[None, :]
    msk = (pcol <= ccol - 384).astype(NP_BF16)

    in_maps = []
    for c in range(NCORES):
        q0 = c * CD
        wq = Wa[:, q0 : q0 + CD]
        wk = Wa[:, D + q0 : D + q0 + CD]
        wv = Wa[:, 2 * D + q0 : 2 * D + q0 + CD]
        bq = ba[q0 : q0 + CD]
        bk = ba[D + q0 : D + q0 + CD]
        bv = ba[2 * D + q0 : 2 * D + q0 + CD]
        in_maps.append(
            {
                "hsT": hsT,
                "w_qkv": np.ascontiguousarray(
                    np.concatenate([wq, wk, wv], axis=1)
                ).astype(NP_BF16),
                "b_qkv": np.ascontiguousarray(np.stack([bq, bk, bv], axis=1)).astype(
                    np.float32
                ),
                "w_p": np.ascontiguousarray(Wp[q0 : q0 + CD, :], dtype=np.float32),
                "msk": msk,
                "iden_b": np.eye(P).astype(NP_BF16),
                "iden_r": np.eye(P, dtype=np.float32),
                "vones": np.ones((P, NSC)).astype(NP_BF16),
            }
        )
    return in_maps


def run(inputs, trace=False):
    """Build+run the sharded kernel. Returns (full_output, BassKernelResults)."""
    in_maps = prepare_inputs(**inputs)
    nc = _get_nc()
    res = run_bass_kernel_spmd(
        nc, in_maps, core_ids=list(range(NCORES)), trace=trace
    )
    acc = np.zeros((S, D), dtype=np.float32)
    for c in range(NCORES):
        acc += res.results[c]["out"]
    acc += np.asarray(inputs["b_proj"], dtype=np.float32)
    return acc, res


def kernel(**inputs):
    out, _ = run(inputs, trace=False)
    return out
